# revision 1
# baseline (speedup 1.0000x reference)
"""Multi-head attention kernel for Trainium2, sharded one head per NeuronCore.

Math (per head h, batch b):
  q,k,v = W_{q,k,v} @ x        [32, n]   (n = 48*48 = 2304, c = 256)
  S~[j,i] = sum_d k[d,j] q[d,i]          (S transposed: j on partitions)
  P[j,i]  = exp(S~[j,i]) * exp(pos_bias[h].T[j,i])
  O_ext   = [v.T | 1]^T-contracted with P:  O_ext[m,i] = sum_j v_ext[j,m] P[j,i]
            rows 0..31 = unnormalized attn@v (transposed), row 32 = softmax sums
  out_un[c,i] = sum_d w_out[c, h*32+d] * O_ext[d,i]
Host: out = sum_h out_un_h / sums_h + b_out  (softmax normalization commutes
with the linear projection, so it is applied on host after gathering).
"""

import sys

for _p in ("/opt/trn_rl_repo", "/root/.axon_site/_ro/trn_rl_repo"):
    if _p not in sys.path:
        sys.path.append(_p)

import numpy as np
import ml_dtypes

import concourse.bacc as bacc
import concourse.mybir as mybir
import concourse.tile as tile
from concourse import bass_utils

HEADS = 8
D = 32                      # dim per head
SCALE = D ** -0.5
B = 4                       # batch
C = 256                     # channels
N = 2304                    # tokens (48*48)
H = W = 48
NJ = 18                     # 128-row j-chunks
JG = 3                      # j-chunks per ACT group (3 psum banks)
NG = NJ // JG               # groups per (b, i-block)
IBLOCKS = [(0, 512), (512, 512), (1024, 512), (1536, 512), (2048, 256)]

F32 = mybir.dt.float32
F32R = mybir.dt.float32r
BF16 = mybir.dt.bfloat16
EXP = mybir.ActivationFunctionType.Exp


VARIANT = "full"  # "full" | "core" (no O/closing; debug-timing only)
LAG_OVERRIDE = None
# tuning knobs (A/B-tested on hardware)
MULT_GPS_GROUPS = ()       # which mult groups go to GpSimd (512-wide blocks)
XROUND_DVE_BATCHES = (0, 1, 2)  # batches whose x-rounding runs on DVE


def _emit(nc, reps=1):
    x_d = nc.dram_tensor("x", [B, C, N], F32, kind="ExternalInput")
    wq_d = nc.dram_tensor("wq", [C, 96], F32, kind="ExternalInput")
    wk_d = nc.dram_tensor("wk", [C, 96], F32, kind="ExternalInput")
    wv_d = nc.dram_tensor("wv", [C, D], F32, kind="ExternalInput")
    wo_d = nc.dram_tensor("wo", [D, C], F32, kind="ExternalInput")
    eb_d = nc.dram_tensor("expb", [N, N], BF16, kind="ExternalInput")
    out_d = nc.dram_tensor("out_un", [B, C, N], F32, kind="ExternalOutput")
    sums_d = nc.dram_tensor("sums", [B, N], F32, kind="ExternalOutput")

    with tile.TileContext(nc) as tc:
        with (
            tc.tile_pool(name="wpool", bufs=1) as wpool,
            tc.tile_pool(name="qk", bufs=8) as qkpool,
            tc.tile_pool(name="vext", bufs=4) as vpool,
            tc.tile_pool(name="big", bufs=2) as bigpool,
            tc.tile_pool(name="pp", bufs=10) as ppool,
            tc.tile_pool(name="ebpool", bufs=2) as ebpool,
            tc.tile_pool(name="outsb", bufs=3) as outpool,
            tc.tile_pool(name="osb", bufs=2) as opool,
            tc.tile_pool(name="spsum", bufs=2, space="PSUM") as spsum,
            tc.tile_pool(name="psA", bufs=2, space="PSUM") as psA,
        ):
            # ---- weights: DMA fp32, round to f32r on DVE ----
            w_raw = {}
            w_r = {}
            for name, dram, shape in (
                ("wq", wq_d, [128, 2, 96]),
                ("wk", wk_d, [128, 2, 96]),
                ("wv", wv_d, [128, 2, D]),
            ):
                raw = wpool.tile(shape, F32, tag=f"{name}raw")
                nc.sync.dma_start(raw, dram.ap().rearrange("(cc p) m -> p cc m", p=128))
                rnd = wpool.tile(shape, F32R, tag=f"{name}r")
                nc.vector.tensor_copy(rnd, raw)
                w_raw[name] = raw
                w_r[name] = rnd
            wo_raw = wpool.tile([D, C], F32, tag="woraw")
            nc.sync.dma_start(wo_raw, wo_d.ap())
            wo_r = wpool.tile([D, C], F32R, tag="wor")
            nc.vector.tensor_copy(wo_r, wo_raw)

            # ---- phase 0 per batch: load x, round, project q/k/v ----
            q_sb = [None] * B
            k_sb = [None] * B
            v_sb = [None] * B

            def proj_batch(b):
                x_raw = bigpool.tile([128, 2, N], F32, tag="big")
                x_view = x_d.ap()[b].rearrange("(cc p) n -> p cc n", p=128)
                x_r = bigpool.tile([128, 2, N], F32R, tag="big")
                # split per c-chunk: two DMA queues in parallel, and the
                # GpSimd rounding (idle engine, line-rate 1-input copies)
                # starts after the first half lands.
                for cc in range(2):
                    nc.sync.dma_start(x_raw[:, cc, :], x_view[:, cc, :])
                    # batch 0 rounds on the (fast) DVE so the ramp is short;
                    # later batches use the slower but otherwise-idle GpSimd.
                    eng = nc.vector if b in XROUND_DVE_BATCHES else nc.gpsimd
                    eng.tensor_copy(x_r[:, cc, :], x_raw[:, cc, :])

                # q and k replicated 3x along output rows (for PE row-tiling)
                for name, store in (("wk", k_sb), ("wq", q_sb)):
                    dst = qkpool.tile([128, N], F32R, tag="qk")
                    store[b] = dst
                    for ti, islices in ((0, (0, 1, 2)), (1, (3, 4))):
                        pt = spsum.tile([128, 3 * 512], F32, tag="sg")
                        for sl, ic in enumerate(islices):
                            i0, iw = IBLOCKS[ic]
                            for cc in range(2):
                                nc.tensor.matmul(
                                    pt[0:96, sl * 512 : sl * 512 + iw],
                                    w_r[name][:, cc, :],
                                    x_r[:, cc, i0 : i0 + iw],
                                    start=(cc == 0),
                                    stop=(cc == 1),
                                )
                        nw = sum(IBLOCKS[ic][1] for ic in islices)
                        if b < 1:
                            nc.scalar.copy(
                                dst[0:96, ti * 1536 : ti * 1536 + nw], pt[0:96, 0:nw]
                            )
                        else:
                            nc.vector.tensor_copy(
                                dst[0:96, ti * 1536 : ti * 1536 + nw], pt[0:96, 0:nw]
                            )

                # v transposed directly: v_T[n, d] = x^T @ wv_T, 18 chunks
                vext = vpool.tile([128, NJ * (D + 1)], BF16, tag="vext")
                v_sb[b] = vext
                nc.vector.memset(vext, 1.0)
                vt = spsum.tile([128, 3 * 512], F32, tag="sg")
                for jc in range(NJ):
                    for cc in range(2):
                        nc.tensor.matmul(
                            vt[:, jc * D : (jc + 1) * D],
                            x_r[:, cc, jc * 128 : (jc + 1) * 128],
                            w_r["wv"][:, cc, :],
                            start=(cc == 0),
                            stop=(cc == 1),
                        )
                nc.vector.tensor_copy(
                    vext.rearrange("p (jc m) -> p jc m", m=D + 1)[:, :, 0:D],
                    vt.rearrange("p (jc m) -> p jc m", m=D)[:, 0:NJ, :],
                )

            # deferred-emission queue: O matmuls (and the per-(b,ib) closing
            # evac/out-projection) are emitted LAG group-units behind the
            # sim/exp/mul stream, so the in-order PE queue never parks an O
            # matmul (waiting on the DVE multiply) in front of later sims.
            o_queue = []
            LAG = 6 if LAG_OVERRIDE is None else LAG_OVERRIDE

            def flush_o(n):
                for _ in range(n):
                    if o_queue:
                        o_queue.pop(0)()

            def group_layout(iw):
                """Per ACT-group chunk placement in the 3-bank S tile.
                512-wide blocks: 3 chunks, one per bank.  256-wide tail: 6
                chunks, bank-interleaved (offset 512*(c%3) + 256*(c//3)) so
                concurrent PE row-tiles never share a PSUM bank."""
                if iw == 512:
                    return [[(g * 3 + jl, jl, jl * 512) for jl in range(3)]
                            for g in range(6)]
                return [
                    [(g * 6 + c, c % 3, (c % 3) * 512 + (c // 3) * 256)
                     for c in range(6)]
                    for g in range(3)
                ]

            def attn(b, ib, eb_t):
                i0, iw = IBLOCKS[ib]
                o_ps = psA.tile([D + 1, 512], F32, tag="pa")
                for g, chunks in enumerate(group_layout(iw)):
                    s_ps = spsum.tile([128, 3 * 512], F32, tag="sg")
                    for jc, row, off in chunks:
                        nc.tensor.matmul(
                            s_ps[:, off : off + iw],
                            k_sb[b][32 * row : 32 * row + 32, jc * 128 : (jc + 1) * 128],
                            q_sb[b][32 * row : 32 * row + 32, i0 : i0 + iw],
                            start=True,
                            stop=True,
                        )
                    # exp over the 3-bank group, psum -> sbuf bf16.  One P
                    # tile per group so exp/mul/O of different groups carry
                    # no false dependencies.
                    p_t = ppool.tile([128, 3 * 512], BF16, tag="pt")
                    nc.scalar.activation(p_t, s_ps, EXP)
                    # multiply by exp(pos_bias) (bf16 2x mode), in place
                    if VARIANT != "core2":
                        eng = nc.gpsimd if (iw == 512 and g in MULT_GPS_GROUPS) else nc.vector
                        eng.tensor_mul(
                            p_t,
                            p_t,
                            eb_t[:, g * 1536 : (g + 1) * 1536],
                        )

                    if VARIANT == "core":
                        continue

                    def o_thunk(chunks=chunks, p_t=p_t, o_ps=o_ps, b=b, iw=iw):
                        for jc, row, off in chunks:
                            nc.tensor.matmul(
                                o_ps[:, 0:iw],
                                v_sb[b][:, jc * (D + 1) : (jc + 1) * (D + 1)],
                                p_t[:, off : off + iw],
                                start=(jc == 0),
                                stop=(jc == NJ - 1),
                            )

                    o_queue.append(o_thunk)
                    while len(o_queue) > LAG:
                        flush_o(1)

                def closing(b=b, i0=i0, iw=iw, o_ps=o_ps):
                    o_t = opool.tile([D + 1, 512], F32R, tag="ot")
                    nc.vector.tensor_copy(o_t[:, 0:iw], o_ps[:, 0:iw])
                    nc.sync.dma_start(
                        sums_d.ap()[b, i0 : i0 + iw], o_t[D : D + 1, 0:iw].bitcast(F32)
                    )
                    for cc in range(2):
                        op_ps = psA.tile([128, 512], F32, tag="pa")
                        nc.tensor.matmul(
                            op_ps[:, 0:iw],
                            wo_r[:, cc * 128 : (cc + 1) * 128],
                            o_t[0:D, 0:iw],
                            start=True,
                            stop=True,
                        )
                        ev = outpool.tile([128, 512], F32, tag="ev")
                        nc.vector.tensor_copy(ev[:, 0:iw], op_ps[:, 0:iw])
                        nc.sync.dma_start(
                            out_d.ap()[b].rearrange("(cc p) n -> p cc n", p=128)[
                                :, cc, i0 : i0 + iw
                            ],
                            ev[:, 0:iw],
                        )

                if VARIANT != "core":
                    o_queue.append(closing)
                elif ib == len(IBLOCKS) - 1 and b == B - 1:
                    # dummy writes so outputs are bound
                    ev = outpool.tile([128, 512], F32, tag="ev")
                    nc.vector.memset(ev, 0.0)
                    for bb in range(B):
                        nc.sync.dma_start(
                            sums_d.ap()[bb, 0:512], ev[0:1, 0:512]
                        )
                        for cc in range(2):
                            nc.sync.dma_start(
                                out_d.ap()[bb].rearrange("(cc p) n -> p cc n", p=128)[
                                    :, cc, 0:512
                                ],
                                ev,
                            )

            # emission order interleaves projections with attention so the
            # Tile scheduler can overlap them.
            def load_eb(ib):
                i0, iw = IBLOCKS[ib]
                eb_t = ebpool.tile([128, NJ * iw], BF16, tag="eb")
                if iw == 512:
                    nc.sync.dma_start(
                        eb_t.rearrange("p (jc i) -> p jc i", i=iw),
                        eb_d.ap().rearrange("(jc p) i -> p jc i", p=128)[
                            :, :, i0 : i0 + iw
                        ],
                    )
                else:
                    # tail: match the bank-interleaved group layout
                    # chunk c -> offset 512*(c%3) + 256*(c//3)
                    src = eb_d.ap().rearrange(
                        "(gg u v p) i -> p gg u v i", p=128, v=3, u=2
                    )
                    for g in range(3):
                        for u in range(2):
                            nc.sync.dma_start(
                                eb_t[:, g * 1536 : (g + 1) * 1536].rearrange(
                                    "p (v u i) -> p u v i", u=2, i=iw
                                )[:, u],
                                src[:, g, u, :, i0 : i0 + iw],
                            )
                return eb_t

            for _rep in range(reps):
                eb0 = load_eb(0)
                proj_batch(0)
                for ib in range(len(IBLOCKS)):
                    eb_t = eb0 if ib == 0 else load_eb(ib)
                    for b in range(B):
                        # defer each projection until just before its batch's
                        # attention: keeps proj evacuations (DVE) out of the
                        # in-order queues ahead of earlier batches' work.
                        if ib == 0 and b >= 1:
                            proj_batch(b)
                        attn(b, ib, eb_t)
                flush_o(len(o_queue))
                o_queue.clear()
    return nc


_CACHE = {}


def _build(reps=1):
    key = ("nc", reps, VARIANT, MULT_GPS_GROUPS, XROUND_DVE_BATCHES, LAG_OVERRIDE)
    if key not in _CACHE:
        nc = bacc.Bacc("TRN2", target_bir_lowering=False, debug=False, num_devices=HEADS)
        _emit(nc, reps=reps)
        nc.compile()
        _CACHE[key] = nc
    return _CACHE[key]


def _prep_inputs(x, pos_bias, w_qkv, w_out):
    xf = np.ascontiguousarray(x.reshape(B, C, N).astype(np.float32))
    in_maps = []
    for h in range(HEADS):
        wq = np.ascontiguousarray(w_qkv[h * D : (h + 1) * D, :].T) * np.float32(SCALE)
        wk = np.ascontiguousarray(w_qkv[C + h * D : C + (h + 1) * D, :].T)
        wv = np.ascontiguousarray(w_qkv[2 * C + h * D : 2 * C + (h + 1) * D, :].T)
        wo = np.ascontiguousarray(w_out[:, h * D : (h + 1) * D].T)
        eb = np.exp(pos_bias[h].T.astype(np.float32)).astype(ml_dtypes.bfloat16)
        in_maps.append(
            {
                "x": xf,
                "wq": np.ascontiguousarray(np.tile(wq, (1, 3))).astype(np.float32),
                "wk": np.ascontiguousarray(np.tile(wk, (1, 3))).astype(np.float32),
                "wv": wv.astype(np.float32),
                "wo": wo.astype(np.float32),
                "expb": np.ascontiguousarray(eb),
            }
        )
    return in_maps


def _run(inputs, trace=False):
    x = np.asarray(inputs["x"], dtype=np.float32)
    pos_bias = np.asarray(inputs["pos_bias"], dtype=np.float32)
    w_qkv = np.asarray(inputs["w_qkv"], dtype=np.float32)
    w_out = np.asarray(inputs["w_out"], dtype=np.float32)
    b_out = np.asarray(inputs["b_out"], dtype=np.float32)

    nc = _build()
    in_maps = _prep_inputs(x, pos_bias, w_qkv, w_out)
    res = bass_utils.run_bass_kernel_spmd(
        nc, in_maps, core_ids=list(range(HEADS)), trace=trace
    )
    out = np.zeros((B, C, N), dtype=np.float32)
    for h in range(HEADS):
        o = res.results[h]["out_un"]
        s = res.results[h]["sums"]
        out += o / s[:, None, :]
    out += b_out[None, :, None]
    return out.reshape(B, C, H, W).astype(np.float32), res


def kernel(**inputs):
    return _run(inputs)[0]



# revision 23
# speedup vs baseline: 1.1719x; 1.1719x over previous
"""Multi-head attention kernel for Trainium2, sharded one head per NeuronCore.

Math (per head h, batch b):
  q,k,v = W_{q,k,v} @ x        [32, n]   (n = 48*48 = 2304, c = 256)
  S~[j,i] = sum_d k[d,j] q[d,i]          (S transposed: j on partitions)
  P[j,i]  = exp(S~[j,i] + pos_bias[h].T[j,i])
     computed either as exp(S~)*exp(B) (Act exp + DVE mult) or via the
     Schraudolph bit-trick on DVE/GpSimd:
       bf16bits(P) ~= int16(A*S~ + round(A*B + 16256)),  A = 128/ln2
     which fuses the bias add and the exp into one off-Act instruction.
  O_ext: [v.T | 1]^T-contracted with P, 2-way column-tiled over the PE
     (even j-chunks -> psum rows 0..32, odd -> rows 64..96) so the two
     accumulation chains run concurrently on distinct col groups:
       rows 0..31/64..95 = partial attn@v (transposed), rows 32/96 = sums
  out_un[c,i] = sum_r wo2[c, r] * o_t[r, i]   (wo2 = [wo; 0; wo; 0])
Host: out = sum_h out_un_h / (sums0_h + sums1_h) + b_out.
"""

import sys

for _p in ("/opt/trn_rl_repo", "/root/.axon_site/_ro/trn_rl_repo"):
    if _p not in sys.path:
        sys.path.append(_p)

import numpy as np
import ml_dtypes

import concourse.bacc as bacc
import concourse.mybir as mybir
import concourse.tile as tile
from concourse import bass_utils

HEADS = 8
D = 32                      # dim per head
SCALE = D ** -0.5
B = 4                       # batch
C = 256                     # channels
N = 2304                    # tokens (48*48)
H = W = 48
NJ = 18                     # 128-row j-chunks
JG = 3                      # j-chunks per ACT group (3 psum banks)
NG = NJ // JG               # groups per (b, i-block)
IBLOCKS = [(0, 512), (512, 512), (1024, 512), (1536, 512), (2048, 256)]

F32 = mybir.dt.float32
F32R = mybir.dt.float32r
BF16 = mybir.dt.bfloat16
I16 = mybir.dt.int16
EXP = mybir.ActivationFunctionType.Exp
MULT = mybir.AluOpType.mult
ADD = mybir.AluOpType.add

SCH_A = float(128.0 / np.log(2.0))   # Schraudolph scale: bf16 exponent grid
# 127<<7, mean-centered (-7.37) so mixed exact/approx rows carry no relative
# bias in the softmax, +0.5 compensating the device's truncating f32->i16.
SCH_B = 16256.0 - 7.37 + 0.5

VARIANT = "full"
LAG_OVERRIDE = None
# tuning knobs
OUT_EVAC_ENG = "scalar"    # engine for out-projection psum->sbuf evacuation
O_COLTILE = True           # 2-way col-tiled O accumulation
# per (ib, g) group path: 'A' = Act exp + DVE mult, 'B' = Act exp + Pool
# (gpsimd) mult, 'D' = DVE schraudolph (GPSIMD cannot read PSUM, so the
# schraudolph path is DVE-only).
ASSIGN = (
    "DABDAB",
    "ADBDAD",
    "DABDAB",
    "ADBDAD",
    "DAB",
)
QK_EVAC_ENG = "vector"     # engine for q/k psum->sbuf evacuation
OT_EVAC_ENG = "scalar"     # engine for o_t psum->sbuf evacuation


def _emit(nc, reps=1):
    x_d = nc.dram_tensor("x", [B, C, N], BF16, kind="ExternalInput")
    wq_d = nc.dram_tensor("wq", [C, 96], BF16, kind="ExternalInput")
    wk_d = nc.dram_tensor("wk", [C, 96], BF16, kind="ExternalInput")
    wv_d = nc.dram_tensor("wv", [C, D], BF16, kind="ExternalInput")
    wo_d = nc.dram_tensor("wo", [97, C], F32, kind="ExternalInput")
    eb_d = nc.dram_tensor("expb", [N, N], BF16, kind="ExternalInput")
    ebi_d = nc.dram_tensor("ebi", [N, N], I16, kind="ExternalInput")
    out_d = nc.dram_tensor("out_un", [B, C, N], F32, kind="ExternalOutput")
    sums_d = nc.dram_tensor("sums", [B, 2, N], F32, kind="ExternalOutput")

    def path_of(ib, g):
        return ASSIGN[ib][g]

    with tile.TileContext(nc) as tc:
        with (
            tc.tile_pool(name="wpool", bufs=1) as wpool,
            tc.tile_pool(name="qk", bufs=8) as qkpool,
            tc.tile_pool(name="vext", bufs=4) as vpool,
            tc.tile_pool(name="big", bufs=2) as bigpool,
            tc.tile_pool(name="pp", bufs=10) as ppool,
            tc.tile_pool(name="ebpool", bufs=2) as ebpool,
            tc.tile_pool(name="outsb", bufs=3) as outpool,
            tc.tile_pool(name="osb", bufs=2) as opool,
            tc.tile_pool(name="spsum", bufs=2, space="PSUM") as spsum,
            tc.tile_pool(name="psA", bufs=2, space="PSUM") as psA,
        ):
            # ---- weights: bf16 straight from HBM (host pre-converts) ----
            w_r = {}
            for name, dram, shape in (
                ("wq", wq_d, [128, 2, 96]),
                ("wk", wk_d, [128, 2, 96]),
                ("wv", wv_d, [128, 2, D]),
            ):
                raw = wpool.tile(shape, BF16, tag=f"{name}raw")
                nc.sync.dma_start(raw, dram.ap().rearrange("(cc p) m -> p cc m", p=128))
                w_r[name] = raw
            wo_raw = wpool.tile([97, C], F32, tag="woraw")
            nc.sync.dma_start(wo_raw, wo_d.ap())
            wo_r = wpool.tile([97, C], F32R, tag="wor")
            nc.vector.tensor_copy(wo_r, wo_raw)
            zrow = wpool.tile([1, 512], F32, tag="zrow")
            nc.vector.memset(zrow, 0.0)
            # o_t buffers: zero once so the dead partition band (33..63)
            # contributes exact zeros to the K=97 out-projection.
            for _ in range(2):
                t = opool.tile([128, 512], F32R, tag="ot")
                nc.vector.memset(t.bitcast(F32), 0.0)

            # ---- phase 0 per batch: load x, project q/k/v ----
            q_sb = [None] * B
            k_sb = [None] * B
            v_sb = [None] * B

            def proj_batch(b):
                x_r = bigpool.tile([128, 2, N], BF16, tag="big")
                x_view = x_d.ap()[b].rearrange("(cc p) n -> p cc n", p=128)
                for cc in range(2):
                    nc.sync.dma_start(x_r[:, cc, :], x_view[:, cc, :])

                # q and k replicated 3x along output rows (for PE row-tiling)
                for name, store in (("wk", k_sb), ("wq", q_sb)):
                    dst = qkpool.tile([128, N], F32R, tag="qk")
                    store[b] = dst
                    for ti, islices in ((0, (0, 1, 2)), (1, (3, 4))):
                        pt = spsum.tile([128, 3 * 512], F32, tag="sg")
                        for sl, ic in enumerate(islices):
                            i0, iw = IBLOCKS[ic]
                            for cc in range(2):
                                nc.tensor.matmul(
                                    pt[0:96, sl * 512 : sl * 512 + iw],
                                    w_r[name][:, cc, :],
                                    x_r[:, cc, i0 : i0 + iw],
                                    start=(cc == 0),
                                    stop=(cc == 1),
                                )
                        nw = sum(IBLOCKS[ic][1] for ic in islices)
                        eng = nc.scalar if b < 1 else getattr(nc, QK_EVAC_ENG)
                        if eng is nc.scalar:
                            eng.copy(
                                dst[0:96, ti * 1536 : ti * 1536 + nw], pt[0:96, 0:nw]
                            )
                        else:
                            eng.tensor_copy(
                                dst[0:96, ti * 1536 : ti * 1536 + nw], pt[0:96, 0:nw]
                            )

                # v transposed directly: v_T[n, d] = x^T @ wv_T, 18 chunks
                vext = vpool.tile([128, NJ * (D + 1)], BF16, tag="vext")
                v_sb[b] = vext
                nc.vector.memset(vext, 1.0)
                vt = spsum.tile([128, 3 * 512], F32, tag="sg")
                for jc in range(NJ):
                    for cc in range(2):
                        nc.tensor.matmul(
                            vt[:, jc * D : (jc + 1) * D],
                            x_r[:, cc, jc * 128 : (jc + 1) * 128],
                            w_r["wv"][:, cc, :],
                            start=(cc == 0),
                            stop=(cc == 1),
                        )
                nc.vector.tensor_copy(
                    vext.rearrange("p (jc m) -> p jc m", m=D + 1)[:, :, 0:D],
                    vt.rearrange("p (jc m) -> p jc m", m=D)[:, 0:NJ, :],
                )

            # deferred-emission queue for O matmuls + closings (see baseline)
            o_queue = []
            LAG = 6 if LAG_OVERRIDE is None else LAG_OVERRIDE

            def flush_o(n):
                for _ in range(n):
                    if o_queue:
                        o_queue.pop(0)()

            def group_layout(iw):
                """Per ACT-group chunk placement in the 3-bank S tile."""
                if iw == 512:
                    return [[(g * 3 + jl, jl, jl * 512) for jl in range(3)]
                            for g in range(6)]
                return [
                    [(g * 6 + c, c % 3, (c % 3) * 512 + (c // 3) * 256)
                     for c in range(6)]
                    for g in range(3)
                ]

            def attn(b, ib, eb_t):
                i0, iw = IBLOCKS[ib]
                o_ps = psA.tile([128, 512], F32, tag="pa")
                for g, chunks in enumerate(group_layout(iw)):
                    path = path_of(ib, g)
                    s_ps = spsum.tile([128, 3 * 512], F32, tag="sg")
                    for jc, row, off in chunks:
                        nc.tensor.matmul(
                            s_ps[:, off : off + iw],
                            k_sb[b][32 * row : 32 * row + 32, jc * 128 : (jc + 1) * 128],
                            q_sb[b][32 * row : 32 * row + 32, i0 : i0 + iw],
                            start=True,
                            stop=True,
                        )
                    p_t = ppool.tile([128, 3 * 512], BF16, tag="pt")
                    if path == "D":
                        # Schraudolph: bf16bits = int16(A*S + ebi), fused
                        # bias add + exp approx, off the Act engine.
                        nc.vector.scalar_tensor_tensor(
                            p_t.bitcast(I16),
                            s_ps,
                            SCH_A,
                            eb_t.bitcast(I16)[:, g * 1536 : (g + 1) * 1536],
                            MULT,
                            ADD,
                        )
                    else:
                        # exp on Act (psum -> sbuf bf16), then * exp(B)
                        nc.scalar.activation(p_t, s_ps, EXP)
                        eng = nc.vector if path == "A" else nc.gpsimd
                        eng.tensor_mul(
                            p_t, p_t, eb_t[:, g * 1536 : (g + 1) * 1536]
                        )

                    if VARIANT == "core":
                        continue

                    def o_thunk(chunks=chunks, p_t=p_t, o_ps=o_ps, b=b, iw=iw):
                        for jc, row, off in chunks:
                            if O_COLTILE:
                                base = 64 * (jc % 2)
                                nc.tensor.matmul(
                                    o_ps[base : base + D + 1, 0:iw],
                                    v_sb[b][:, jc * (D + 1) : (jc + 1) * (D + 1)],
                                    p_t[:, off : off + iw],
                                    start=(jc < 2),
                                    stop=(jc >= NJ - 2),
                                )
                            else:
                                nc.tensor.matmul(
                                    o_ps[0 : D + 1, 0:iw],
                                    v_sb[b][:, jc * (D + 1) : (jc + 1) * (D + 1)],
                                    p_t[:, off : off + iw],
                                    start=(jc == 0),
                                    stop=(jc == NJ - 1),
                                )

                    o_queue.append(o_thunk)
                    while len(o_queue) > LAG:
                        flush_o(1)

                def closing(b=b, i0=i0, iw=iw, o_ps=o_ps):
                    nrow = 97 if O_COLTILE else D + 1
                    o_t = opool.tile([128, 512], F32R, tag="ot")
                    ev_eng = getattr(nc, OT_EVAC_ENG)

                    def evac(dst, src):
                        if ev_eng is nc.scalar:
                            ev_eng.copy(dst, src)
                        else:
                            ev_eng.tensor_copy(dst, src)

                    if O_COLTILE:
                        evac(o_t[0 : D + 1, 0:iw], o_ps[0 : D + 1, 0:iw])
                        evac(o_t[64 : 64 + D + 1, 0:iw], o_ps[64 : 64 + D + 1, 0:iw])
                        nc.sync.dma_start(
                            sums_d.ap()[b, 0, i0 : i0 + iw],
                            o_t[D : D + 1, 0:iw].bitcast(F32),
                        )
                        nc.sync.dma_start(
                            sums_d.ap()[b, 1, i0 : i0 + iw],
                            o_t[96:97, 0:iw].bitcast(F32),
                        )
                    else:
                        evac(o_t[0 : D + 1, 0:iw], o_ps[0 : D + 1, 0:iw])
                        nc.sync.dma_start(
                            sums_d.ap()[b, 0, i0 : i0 + iw],
                            o_t[D : D + 1, 0:iw].bitcast(F32),
                        )
                        nc.sync.dma_start(
                            sums_d.ap()[b, 1, i0 : i0 + iw], zrow[:, 0:iw]
                        )

                    out_view = out_d.ap()[b].rearrange("(cc p) n -> p cc n", p=128)
                    for cc in range(2):
                        op_ps = psA.tile([128, 512], F32, tag="pa")
                        nc.tensor.matmul(
                            op_ps[:, 0:iw],
                            wo_r[0:nrow, cc * 128 : (cc + 1) * 128],
                            o_t[0:nrow, 0:iw],
                            start=True,
                            stop=True,
                        )
                        ev = outpool.tile([128, 512], F32, tag="ev")
                        oe = getattr(nc, OUT_EVAC_ENG)
                        if oe is nc.scalar:
                            oe.copy(ev[:, 0:iw], op_ps[:, 0:iw])
                        else:
                            oe.tensor_copy(ev[:, 0:iw], op_ps[:, 0:iw])
                        nc.sync.dma_start(
                            out_view[:, cc, i0 : i0 + iw], ev[:, 0:iw]
                        )

                if VARIANT != "core":
                    o_queue.append(closing)

            # eb loading: ACT groups read exp(B) bf16 from expb, schraudolph
            # groups read int16(A*B + 16256) from ebi; both are 2-byte so
            # they share the eb_t tile (i16 slices via bitcast).
            def load_eb(ib):
                i0, iw = IBLOCKS[ib]
                eb_t = ebpool.tile([128, NJ * iw], BF16, tag="eb")
                if iw == 512:
                    srcs = {
                        "A": eb_d.ap().rearrange("(jc p) i -> p jc i", p=128),
                        "S": ebi_d.ap().rearrange("(jc p) i -> p jc i", p=128),
                    }
                    view = eb_t.rearrange("p (jc i) -> p jc i", i=iw)
                    iview = eb_t.bitcast(I16).rearrange("p (jc i) -> p jc i", i=iw)
                    # batch contiguous same-form group runs into single DMAs
                    runs = []
                    for g in range(6):
                        form = "S" if path_of(ib, g) == "D" else "A"
                        if runs and runs[-1][0] == form:
                            runs[-1][2] = 3 * (g + 1)
                        else:
                            runs.append([form, 3 * g, 3 * (g + 1)])
                    for form, lo, hi in runs:
                        dst = view if form == "A" else iview
                        nc.sync.dma_start(
                            dst[:, lo:hi, :],
                            srcs[form][:, lo:hi, i0 : i0 + iw],
                        )
                else:
                    # tail: match the bank-interleaved group layout
                    srcs = {
                        "A": eb_d.ap().rearrange(
                            "(gg u v p) i -> p gg u v i", p=128, v=3, u=2
                        ),
                        "S": ebi_d.ap().rearrange(
                            "(gg u v p) i -> p gg u v i", p=128, v=3, u=2
                        ),
                    }
                    for g in range(3):
                        form = "A" if path_of(ib, g) != "D" else "S"
                        gsl = eb_t[:, g * 1536 : (g + 1) * 1536]
                        if form == "S":
                            gsl = gsl.bitcast(I16)
                        for u in range(2):
                            nc.sync.dma_start(
                                gsl.rearrange(
                                    "p (v u i) -> p u v i", u=2, i=iw
                                )[:, u],
                                srcs[form][:, g, u, :, i0 : i0 + iw],
                            )
                return eb_t

            for _rep in range(reps):
                eb0 = load_eb(0)
                proj_batch(0)
                for ib in range(len(IBLOCKS)):
                    eb_t = eb0 if ib == 0 else load_eb(ib)
                    for b in range(B):
                        if ib == 0 and b >= 1:
                            proj_batch(b)
                        attn(b, ib, eb_t)
                flush_o(len(o_queue))
                o_queue.clear()
    return nc


_CACHE = {}


def _build(reps=1):
    key = ("nc", reps, VARIANT, ASSIGN, OUT_EVAC_ENG,
           O_COLTILE, QK_EVAC_ENG, OT_EVAC_ENG, LAG_OVERRIDE)
    if key not in _CACHE:
        nc = bacc.Bacc("TRN2", target_bir_lowering=False, debug=False, num_devices=HEADS)
        _emit(nc, reps=reps)
        nc.compile()
        _CACHE[key] = nc
    return _CACHE[key]


def _prep_inputs(x, pos_bias, w_qkv, w_out):
    bf16 = ml_dtypes.bfloat16
    xf = np.ascontiguousarray(x.reshape(B, C, N).astype(bf16))
    in_maps = []
    for h in range(HEADS):
        wq = np.ascontiguousarray(w_qkv[h * D : (h + 1) * D, :].T) * np.float32(SCALE)
        wk = np.ascontiguousarray(w_qkv[C + h * D : C + (h + 1) * D, :].T)
        wv = np.ascontiguousarray(w_qkv[2 * C + h * D : 2 * C + (h + 1) * D, :].T)
        wo = np.ascontiguousarray(w_out[:, h * D : (h + 1) * D].T)  # [32, 256]
        wo2 = np.zeros((97, C), dtype=np.float32)
        wo2[0:D] = wo
        wo2[64 : 64 + D] = wo
        bT = pos_bias[h].T.astype(np.float64)
        eb = np.exp(bT).astype(bf16)
        ebi = np.round(SCH_A * bT + SCH_B).astype(np.int16)
        in_maps.append(
            {
                "x": xf,
                "wq": np.ascontiguousarray(np.tile(wq, (1, 3))).astype(bf16),
                "wk": np.ascontiguousarray(np.tile(wk, (1, 3))).astype(bf16),
                "wv": wv.astype(bf16),
                "wo": wo2,
                "expb": np.ascontiguousarray(eb),
                "ebi": np.ascontiguousarray(ebi),
            }
        )
    return in_maps


def _run(inputs, trace=False):
    x = np.asarray(inputs["x"], dtype=np.float32)
    pos_bias = np.asarray(inputs["pos_bias"], dtype=np.float32)
    w_qkv = np.asarray(inputs["w_qkv"], dtype=np.float32)
    w_out = np.asarray(inputs["w_out"], dtype=np.float32)
    b_out = np.asarray(inputs["b_out"], dtype=np.float32)

    nc = _build()
    in_maps = _prep_inputs(x, pos_bias, w_qkv, w_out)
    res = bass_utils.run_bass_kernel_spmd(
        nc, in_maps, core_ids=list(range(HEADS)), trace=trace
    )
    out = np.zeros((B, C, N), dtype=np.float32)
    for h in range(HEADS):
        o = res.results[h]["out_un"]
        s = res.results[h]["sums"]
        out += o / (s[:, 0][:, None, :] + s[:, 1][:, None, :])
    out += b_out[None, :, None]
    return out.reshape(B, C, H, W).astype(np.float32), res


def kernel(**inputs):
    return _run(inputs)[0]


# revision 26
# speedup vs baseline: 1.1983x; 1.0225x over previous
"""Multi-head attention kernel for Trainium2, sharded one head per NeuronCore.

Math (per head h, batch b):
  q,k,v = W_{q,k,v} @ x        [32, n]   (n = 48*48 = 2304, c = 256)
  S~[j,i] = sum_d k[d,j] q[d,i]          (S transposed: j on partitions)
  P[j,i]  = exp(S~[j,i] + pos_bias[h].T[j,i])
     computed either as exp(S~)*exp(B) (Act exp + DVE mult) or via the
     Schraudolph bit-trick on DVE/GpSimd:
       bf16bits(P) ~= int16(A*S~ + round(A*B + 16256)),  A = 128/ln2
     which fuses the bias add and the exp into one off-Act instruction.
  O_ext: [v.T | 1]^T-contracted with P, 2-way column-tiled over the PE
     (even j-chunks -> psum rows 0..32, odd -> rows 64..96) so the two
     accumulation chains run concurrently on distinct col groups:
       rows 0..31/64..95 = partial attn@v (transposed), rows 32/96 = sums
  out_un[c,i] = sum_r wo2[c, r] * o_t[r, i]   (wo2 = [wo; 0; wo; 0])
Host: out = sum_h out_un_h / (sums0_h + sums1_h) + b_out.
"""

import sys

for _p in ("/opt/trn_rl_repo", "/root/.axon_site/_ro/trn_rl_repo"):
    if _p not in sys.path:
        sys.path.append(_p)

import numpy as np
import ml_dtypes

import concourse.bacc as bacc
import concourse.mybir as mybir
import concourse.tile as tile
from concourse import bass_utils

HEADS = 8
D = 32                      # dim per head
SCALE = D ** -0.5
B = 4                       # batch
C = 256                     # channels
N = 2304                    # tokens (48*48)
H = W = 48
NJ = 18                     # 128-row j-chunks
JG = 3                      # j-chunks per ACT group (3 psum banks)
NG = NJ // JG               # groups per (b, i-block)
IBLOCKS = [(0, 512), (512, 512), (1024, 512), (1536, 512), (2048, 256)]

F32 = mybir.dt.float32
F32R = mybir.dt.float32r
BF16 = mybir.dt.bfloat16
I16 = mybir.dt.int16
EXP = mybir.ActivationFunctionType.Exp
MULT = mybir.AluOpType.mult
ADD = mybir.AluOpType.add

SCH_A = float(128.0 / np.log(2.0))   # Schraudolph scale: bf16 exponent grid
# 127<<7, mean-centered (-7.37) so mixed exact/approx rows carry no relative
# bias in the softmax, +0.5 compensating the device's truncating f32->i16.
SCH_B = 16256.0 - 7.37 + 0.5

VARIANT = "full"
LAG_OVERRIDE = 12
# tuning knobs
OUT_EVAC_ENG = "scalar"    # engine for out-projection psum->sbuf evacuation
O_COLTILE = True           # 2-way col-tiled O accumulation
# per (ib, g) group path: 'A' = Act exp + DVE mult, 'B' = Act exp + Pool
# (gpsimd) mult, 'D' = DVE schraudolph (GPSIMD cannot read PSUM, so the
# schraudolph path is DVE-only).  B groups lead each i-block: their
# two-engine chain has the longest latency, so give them the most slack
# before their deferred O matmuls reach the head of the PE queue.
ASSIGN = (
    "BDBDAA",
    "BDADAD",
    "BDBDAA",
    "BDADAD",
    "BDA",
)
QK_EVAC_ENG = "vector"     # engine for q/k psum->sbuf evacuation
OT_EVAC_ENG = "scalar"     # engine for o_t psum->sbuf evacuation


def _emit(nc, reps=1):
    x_d = nc.dram_tensor("x", [B, C, N], BF16, kind="ExternalInput")
    wq_d = nc.dram_tensor("wq", [C, 96], BF16, kind="ExternalInput")
    wk_d = nc.dram_tensor("wk", [C, 96], BF16, kind="ExternalInput")
    wv_d = nc.dram_tensor("wv", [C, D], BF16, kind="ExternalInput")
    wo_d = nc.dram_tensor("wo", [97, C], F32, kind="ExternalInput")
    eb_d = nc.dram_tensor("expb", [N, N], BF16, kind="ExternalInput")
    ebi_d = nc.dram_tensor("ebi", [N, N], I16, kind="ExternalInput")
    out_d = nc.dram_tensor("out_un", [B, C, N], F32, kind="ExternalOutput")
    sums_d = nc.dram_tensor("sums", [B, 2, N], F32, kind="ExternalOutput")

    def path_of(ib, g):
        return ASSIGN[ib][g]

    with tile.TileContext(nc) as tc:
        with (
            tc.tile_pool(name="wpool", bufs=1) as wpool,
            tc.tile_pool(name="qk", bufs=8) as qkpool,
            tc.tile_pool(name="vext", bufs=4) as vpool,
            tc.tile_pool(name="big", bufs=2) as bigpool,
            tc.tile_pool(name="pp", bufs=15) as ppool,
            tc.tile_pool(name="ebpool", bufs=2) as ebpool,
            tc.tile_pool(name="outsb", bufs=3) as outpool,
            tc.tile_pool(name="osb", bufs=2) as opool,
            tc.tile_pool(name="spsum", bufs=2, space="PSUM") as spsum,
            tc.tile_pool(name="psA", bufs=2, space="PSUM") as psA,
        ):
            # ---- weights: bf16 straight from HBM (host pre-converts) ----
            w_r = {}
            for name, dram, shape in (
                ("wq", wq_d, [128, 2, 96]),
                ("wk", wk_d, [128, 2, 96]),
                ("wv", wv_d, [128, 2, D]),
            ):
                raw = wpool.tile(shape, BF16, tag=f"{name}raw")
                nc.sync.dma_start(raw, dram.ap().rearrange("(cc p) m -> p cc m", p=128))
                w_r[name] = raw
            wo_raw = wpool.tile([97, C], F32, tag="woraw")
            nc.sync.dma_start(wo_raw, wo_d.ap())
            wo_r = wpool.tile([97, C], F32R, tag="wor")
            nc.vector.tensor_copy(wo_r, wo_raw)
            zrow = wpool.tile([1, 512], F32, tag="zrow")
            nc.vector.memset(zrow, 0.0)
            # o_t buffers: zero once so the dead partition band (33..63)
            # contributes exact zeros to the K=97 out-projection.
            for _ in range(2):
                t = opool.tile([128, 512], F32R, tag="ot")
                nc.vector.memset(t.bitcast(F32), 0.0)

            # ---- phase 0 per batch: load x, project q/k/v ----
            q_sb = [None] * B
            k_sb = [None] * B
            v_sb = [None] * B

            def proj_batch(b):
                x_r = bigpool.tile([128, 2, N], BF16, tag="big")
                x_view = x_d.ap()[b].rearrange("(cc p) n -> p cc n", p=128)
                for cc in range(2):
                    nc.sync.dma_start(x_r[:, cc, :], x_view[:, cc, :])

                # q and k replicated 3x along output rows (for PE row-tiling)
                for name, store in (("wk", k_sb), ("wq", q_sb)):
                    dst = qkpool.tile([128, N], F32R, tag="qk")
                    store[b] = dst
                    for ti, islices in ((0, (0, 1, 2)), (1, (3, 4))):
                        pt = spsum.tile([128, 3 * 512], F32, tag="sg")
                        for sl, ic in enumerate(islices):
                            i0, iw = IBLOCKS[ic]
                            for cc in range(2):
                                nc.tensor.matmul(
                                    pt[0:96, sl * 512 : sl * 512 + iw],
                                    w_r[name][:, cc, :],
                                    x_r[:, cc, i0 : i0 + iw],
                                    start=(cc == 0),
                                    stop=(cc == 1),
                                )
                        nw = sum(IBLOCKS[ic][1] for ic in islices)
                        eng = nc.scalar if b < 1 else getattr(nc, QK_EVAC_ENG)
                        if eng is nc.scalar:
                            eng.copy(
                                dst[0:96, ti * 1536 : ti * 1536 + nw], pt[0:96, 0:nw]
                            )
                        else:
                            eng.tensor_copy(
                                dst[0:96, ti * 1536 : ti * 1536 + nw], pt[0:96, 0:nw]
                            )

                # v transposed directly: v_T[n, d] = x^T @ wv_T, 18 chunks
                vext = vpool.tile([128, NJ * (D + 1)], BF16, tag="vext")
                v_sb[b] = vext
                nc.vector.memset(vext, 1.0)
                vt = spsum.tile([128, 3 * 512], F32, tag="sg")
                for jc in range(NJ):
                    for cc in range(2):
                        nc.tensor.matmul(
                            vt[:, jc * D : (jc + 1) * D],
                            x_r[:, cc, jc * 128 : (jc + 1) * 128],
                            w_r["wv"][:, cc, :],
                            start=(cc == 0),
                            stop=(cc == 1),
                        )
                nc.vector.tensor_copy(
                    vext.rearrange("p (jc m) -> p jc m", m=D + 1)[:, :, 0:D],
                    vt.rearrange("p (jc m) -> p jc m", m=D)[:, 0:NJ, :],
                )

            # deferred-emission queue for O matmuls + closings (see baseline)
            o_queue = []
            LAG = 6 if LAG_OVERRIDE is None else LAG_OVERRIDE

            def flush_o(n):
                for _ in range(n):
                    if o_queue:
                        o_queue.pop(0)()

            def group_layout(iw):
                """Per ACT-group chunk placement in the 3-bank S tile."""
                if iw == 512:
                    return [[(g * 3 + jl, jl, jl * 512) for jl in range(3)]
                            for g in range(6)]
                return [
                    [(g * 6 + c, c % 3, (c % 3) * 512 + (c // 3) * 256)
                     for c in range(6)]
                    for g in range(3)
                ]

            def attn(b, ib, eb_t):
                i0, iw = IBLOCKS[ib]
                o_ps = psA.tile([128, 512], F32, tag="pa")
                for g, chunks in enumerate(group_layout(iw)):
                    path = path_of(ib, g)
                    s_ps = spsum.tile([128, 3 * 512], F32, tag="sg")
                    for jc, row, off in chunks:
                        nc.tensor.matmul(
                            s_ps[:, off : off + iw],
                            k_sb[b][32 * row : 32 * row + 32, jc * 128 : (jc + 1) * 128],
                            q_sb[b][32 * row : 32 * row + 32, i0 : i0 + iw],
                            start=True,
                            stop=True,
                        )
                    p_t = ppool.tile([128, 3 * 512], BF16, tag="pt")
                    if path == "D":
                        # Schraudolph: bf16bits = int16(A*S + ebi), fused
                        # bias add + exp approx, off the Act engine.
                        nc.vector.scalar_tensor_tensor(
                            p_t.bitcast(I16),
                            s_ps,
                            SCH_A,
                            eb_t.bitcast(I16)[:, g * 1536 : (g + 1) * 1536],
                            MULT,
                            ADD,
                        )
                    else:
                        # exp on Act (psum -> sbuf bf16), then * exp(B)
                        nc.scalar.activation(p_t, s_ps, EXP)
                        eng = nc.vector if path == "A" else nc.gpsimd
                        eng.tensor_mul(
                            p_t, p_t, eb_t[:, g * 1536 : (g + 1) * 1536]
                        )

                    if VARIANT == "core":
                        continue

                    def o_thunk(chunks=chunks, p_t=p_t, o_ps=o_ps, b=b, iw=iw):
                        for jc, row, off in chunks:
                            if O_COLTILE:
                                base = 64 * (jc % 2)
                                nc.tensor.matmul(
                                    o_ps[base : base + D + 1, 0:iw],
                                    v_sb[b][:, jc * (D + 1) : (jc + 1) * (D + 1)],
                                    p_t[:, off : off + iw],
                                    start=(jc < 2),
                                    stop=(jc >= NJ - 2),
                                )
                            else:
                                nc.tensor.matmul(
                                    o_ps[0 : D + 1, 0:iw],
                                    v_sb[b][:, jc * (D + 1) : (jc + 1) * (D + 1)],
                                    p_t[:, off : off + iw],
                                    start=(jc == 0),
                                    stop=(jc == NJ - 1),
                                )

                    o_queue.append(o_thunk)
                    while len(o_queue) > LAG:
                        flush_o(1)

                def closing(b=b, i0=i0, iw=iw, o_ps=o_ps):
                    nrow = 97 if O_COLTILE else D + 1
                    o_t = opool.tile([128, 512], F32R, tag="ot")
                    ev_eng = getattr(nc, OT_EVAC_ENG)

                    def evac(dst, src):
                        if ev_eng is nc.scalar:
                            ev_eng.copy(dst, src)
                        else:
                            ev_eng.tensor_copy(dst, src)

                    if O_COLTILE:
                        evac(o_t[0 : D + 1, 0:iw], o_ps[0 : D + 1, 0:iw])
                        evac(o_t[64 : 64 + D + 1, 0:iw], o_ps[64 : 64 + D + 1, 0:iw])
                        nc.sync.dma_start(
                            sums_d.ap()[b, 0, i0 : i0 + iw],
                            o_t[D : D + 1, 0:iw].bitcast(F32),
                        )
                        nc.sync.dma_start(
                            sums_d.ap()[b, 1, i0 : i0 + iw],
                            o_t[96:97, 0:iw].bitcast(F32),
                        )
                    else:
                        evac(o_t[0 : D + 1, 0:iw], o_ps[0 : D + 1, 0:iw])
                        nc.sync.dma_start(
                            sums_d.ap()[b, 0, i0 : i0 + iw],
                            o_t[D : D + 1, 0:iw].bitcast(F32),
                        )
                        nc.sync.dma_start(
                            sums_d.ap()[b, 1, i0 : i0 + iw], zrow[:, 0:iw]
                        )

                    out_view = out_d.ap()[b].rearrange("(cc p) n -> p cc n", p=128)
                    for cc in range(2):
                        op_ps = psA.tile([128, 512], F32, tag="pa")
                        nc.tensor.matmul(
                            op_ps[:, 0:iw],
                            wo_r[0:nrow, cc * 128 : (cc + 1) * 128],
                            o_t[0:nrow, 0:iw],
                            start=True,
                            stop=True,
                        )
                        ev = outpool.tile([128, 512], F32, tag="ev")
                        oe = getattr(nc, OUT_EVAC_ENG)
                        if oe is nc.scalar:
                            oe.copy(ev[:, 0:iw], op_ps[:, 0:iw])
                        else:
                            oe.tensor_copy(ev[:, 0:iw], op_ps[:, 0:iw])
                        nc.sync.dma_start(
                            out_view[:, cc, i0 : i0 + iw], ev[:, 0:iw]
                        )

                if VARIANT != "core":
                    o_queue.append(closing)
                elif ib == len(IBLOCKS) - 1 and b == B - 1:
                    # dummy writes so outputs are bound
                    ev = outpool.tile([128, 512], F32, tag="ev")
                    nc.vector.memset(ev, 0.0)
                    for bb in range(B):
                        nc.sync.dma_start(sums_d.ap()[bb, 0, 0:512], ev[0:1, 0:512])
                        nc.sync.dma_start(sums_d.ap()[bb, 1, 0:512], ev[0:1, 0:512])
                        for cc in range(2):
                            nc.sync.dma_start(
                                out_d.ap()[bb].rearrange("(cc p) n -> p cc n", p=128)[
                                    :, cc, 0:512
                                ],
                                ev,
                            )

            # eb loading: ACT groups read exp(B) bf16 from expb, schraudolph
            # groups read int16(A*B + 16256) from ebi; both are 2-byte so
            # they share the eb_t tile (i16 slices via bitcast).
            def load_eb(ib):
                i0, iw = IBLOCKS[ib]
                eb_t = ebpool.tile([128, NJ * iw], BF16, tag="eb")
                if iw == 512:
                    srcs = {
                        "A": eb_d.ap().rearrange("(jc p) i -> p jc i", p=128),
                        "S": ebi_d.ap().rearrange("(jc p) i -> p jc i", p=128),
                    }
                    view = eb_t.rearrange("p (jc i) -> p jc i", i=iw)
                    iview = eb_t.bitcast(I16).rearrange("p (jc i) -> p jc i", i=iw)
                    # batch contiguous same-form group runs into single DMAs
                    runs = []
                    for g in range(6):
                        form = "S" if path_of(ib, g) == "D" else "A"
                        if runs and runs[-1][0] == form:
                            runs[-1][2] = 3 * (g + 1)
                        else:
                            runs.append([form, 3 * g, 3 * (g + 1)])
                    for form, lo, hi in runs:
                        dst = view if form == "A" else iview
                        nc.sync.dma_start(
                            dst[:, lo:hi, :],
                            srcs[form][:, lo:hi, i0 : i0 + iw],
                        )
                else:
                    # tail: match the bank-interleaved group layout
                    srcs = {
                        "A": eb_d.ap().rearrange(
                            "(gg u v p) i -> p gg u v i", p=128, v=3, u=2
                        ),
                        "S": ebi_d.ap().rearrange(
                            "(gg u v p) i -> p gg u v i", p=128, v=3, u=2
                        ),
                    }
                    for g in range(3):
                        form = "A" if path_of(ib, g) != "D" else "S"
                        gsl = eb_t[:, g * 1536 : (g + 1) * 1536]
                        if form == "S":
                            gsl = gsl.bitcast(I16)
                        for u in range(2):
                            nc.sync.dma_start(
                                gsl.rearrange(
                                    "p (v u i) -> p u v i", u=2, i=iw
                                )[:, u],
                                srcs[form][:, g, u, :, i0 : i0 + iw],
                            )
                return eb_t

            for _rep in range(reps):
                eb0 = load_eb(0)
                proj_batch(0)
                for ib in range(len(IBLOCKS)):
                    eb_t = eb0 if ib == 0 else load_eb(ib)
                    for b in range(B):
                        if ib == 0 and b >= 1:
                            proj_batch(b)
                        attn(b, ib, eb_t)
                flush_o(len(o_queue))
                o_queue.clear()
    return nc


_CACHE = {}


def _build(reps=1):
    key = ("nc", reps, VARIANT, ASSIGN, OUT_EVAC_ENG,
           O_COLTILE, QK_EVAC_ENG, OT_EVAC_ENG, LAG_OVERRIDE)
    if key not in _CACHE:
        nc = bacc.Bacc("TRN2", target_bir_lowering=False, debug=False, num_devices=HEADS)
        _emit(nc, reps=reps)
        nc.compile()
        _CACHE[key] = nc
    return _CACHE[key]


def _prep_inputs(x, pos_bias, w_qkv, w_out):
    bf16 = ml_dtypes.bfloat16
    xf = np.ascontiguousarray(x.reshape(B, C, N).astype(bf16))
    in_maps = []
    for h in range(HEADS):
        wq = np.ascontiguousarray(w_qkv[h * D : (h + 1) * D, :].T) * np.float32(SCALE)
        wk = np.ascontiguousarray(w_qkv[C + h * D : C + (h + 1) * D, :].T)
        wv = np.ascontiguousarray(w_qkv[2 * C + h * D : 2 * C + (h + 1) * D, :].T)
        wo = np.ascontiguousarray(w_out[:, h * D : (h + 1) * D].T)  # [32, 256]
        wo2 = np.zeros((97, C), dtype=np.float32)
        wo2[0:D] = wo
        wo2[64 : 64 + D] = wo
        bT = pos_bias[h].T.astype(np.float64)
        eb = np.exp(bT).astype(bf16)
        ebi = np.round(SCH_A * bT + SCH_B).astype(np.int16)
        in_maps.append(
            {
                "x": xf,
                "wq": np.ascontiguousarray(np.tile(wq, (1, 3))).astype(bf16),
                "wk": np.ascontiguousarray(np.tile(wk, (1, 3))).astype(bf16),
                "wv": wv.astype(bf16),
                "wo": wo2,
                "expb": np.ascontiguousarray(eb),
                "ebi": np.ascontiguousarray(ebi),
            }
        )
    return in_maps


def _run(inputs, trace=False):
    x = np.asarray(inputs["x"], dtype=np.float32)
    pos_bias = np.asarray(inputs["pos_bias"], dtype=np.float32)
    w_qkv = np.asarray(inputs["w_qkv"], dtype=np.float32)
    w_out = np.asarray(inputs["w_out"], dtype=np.float32)
    b_out = np.asarray(inputs["b_out"], dtype=np.float32)

    nc = _build()
    in_maps = _prep_inputs(x, pos_bias, w_qkv, w_out)
    res = bass_utils.run_bass_kernel_spmd(
        nc, in_maps, core_ids=list(range(HEADS)), trace=trace
    )
    out = np.zeros((B, C, N), dtype=np.float32)
    for h in range(HEADS):
        o = res.results[h]["out_un"]
        s = res.results[h]["sums"]
        out += o / (s[:, 0][:, None, :] + s[:, 1][:, None, :])
    out += b_out[None, :, None]
    return out.reshape(B, C, H, W).astype(np.float32), res


def kernel(**inputs):
    return _run(inputs)[0]
